# revision 1
# baseline (speedup 1.0000x reference)
"""Causal cross-attention (q=k=v projection) kernel for Trainium2, 8 cores.

Problem (hardcoded): B=8, L=2048, D=1024, fp32.
    q = x @ Wq.T; k = enc @ Wq.T; v = k
    s = causal(q @ k.T / sqrt(D)); out = softmax(s) @ v

Sharding: data-parallel over batch, one batch element per NeuronCore, Wq
replicated.  All on-chip compute in bf16 (fp32 PSUM accumulation); the host
pre-transposes inputs so every matmul has its contraction dim on partitions:

per core, with xT/encT/wqT host-prepared ([D,L]/[D,L]/[D,D] bf16):
  QT[h,q] = sum_d wqT[d,h] * xT[d,q]            (lhsT=wqT tile, rhs=xT tile)
  Kkh[k,h] = sum_d encT[d,k] * wqT[d,h]         (lhsT=encT tile, rhs=wqT)
  KT[h,k]  = PE-transpose(Kkh)
  ST[k,q]  = sum_h KT[h,k] * QT[h,q]            (scores, transposed)
  P[k,q]   = exp(ST/32) * causal_mask           (no max-subtraction: |s|<~6)
  rowsum[q]= ones.T @ P                          (PE, accumulated over k chunks)
  OT[h,q]  = sum_k Kkh[k,h] * P[k,q]            (attn @ v, transposed)
  out_t    = OT * (1/rowsum)  broadcast over partitions
Host transposes out_t back to [L, D] per batch.
"""

import sys

for _p in ("/opt/trn_rl_repo",):
    if _p not in sys.path:
        sys.path.insert(0, _p)

import numpy as np
import ml_dtypes

import concourse.bass as bass
import concourse.tile as tile
from concourse import bacc, mybir
from concourse.masks import make_identity, make_upper_triangular

B, L, D = 8, 2048, 1024
P = 128                    # partitions
ND = D // P                # 8 d-chunks
NH = D // P                # 8 h-chunks
NK = L // P                # 16 k-chunks
QW = 512                   # q block width
NJ = L // QW               # 4 q blocks
SCALE = 1.0 / np.sqrt(np.float32(D))   # 1/32

BF16 = mybir.dt.bfloat16
F32 = mybir.dt.float32

_CACHED = {}


def build_program():
    """Build the per-core Bass/Tile program (same for all 8 cores)."""
    # Bacc (not raw Bass): its compile() splits multi-sem waits into
    # EventSemaphore instructions — walrus encodes at most one wait per
    # instruction, so raw Bass programs with Tile-generated multi-waits
    # fail codegen ("Too many sync wait commands").
    nc = bacc.Bacc("TRN2", target_bir_lowering=False, debug=False, num_devices=B)

    xt = nc.declare_dram_parameter("xt", [D, L], BF16, isOutput=False).ap()
    enct = nc.declare_dram_parameter("enct", [D, L], BF16, isOutput=False).ap()
    wqt = nc.declare_dram_parameter("wqt", [D, D], BF16, isOutput=False).ap()
    # separate output tensor per (q-block, h-chunk) store: a single shared
    # output tensor makes Tile chain every store with a cross-queue WAW wait,
    # and 2-wait DMAs don't fit the direct2d descriptor (walrus error).
    outs = [
        [nc.declare_dram_parameter(f"o_{J}_{ch}", [P, QW], F32,
                                   isOutput=True).ap()
         for ch in range(NH)]
        for J in range(NJ)
    ]

    with tile.TileContext(nc) as tc:
        _emit(nc, tc, xt, enct, wqt, outs)
    nc.compile()
    _check_dma_waits(nc)
    return nc


def _check_dma_waits(nc):
    """HWDGE direct2d descriptors encode only ONE wait; multi-wait DMAs make
    walrus fail codegen. SWDGE (gpsimd queue) triggers run on the Pool
    sequencer where walrus can split waits, so only check HWDGE queues."""
    fn = nc.m.functions[0]
    bad = [
        (i.name, i.queue, [(w.ant_name, w.wait_value) for w in i.sync_info.on_wait])
        for blk in fn.blocks for i in blk.instructions
        if type(i).__name__ == "InstDMACopy"
        and "DynamicHW" in getattr(i, "queue", "")
        and len(i.sync_info.on_wait) > 1
    ]
    assert not bad, f"multi-wait HWDGE DMAs: {bad[:4]} (+{len(bad)-4} more)"


def _emit(nc, tc, xt, enct, wqt, outs):
    from contextlib import ExitStack

    ctx = ExitStack()
    consts = ctx.enter_context(tc.tile_pool(name="consts", bufs=1))
    kt_pool = ctx.enter_context(tc.tile_pool(name="ktp", bufs=1))
    kkh_pool = ctx.enter_context(tc.tile_pool(name="kkhp", bufs=1))
    enc_pool = ctx.enter_context(tc.tile_pool(name="encp", bufs=4))
    x_pool = ctx.enter_context(tc.tile_pool(name="xp", bufs=4))
    qt_pool = ctx.enter_context(tc.tile_pool(name="qtp", bufs=2))
    p_pool = ctx.enter_context(tc.tile_pool(name="pp", bufs=18))
    o_pool = ctx.enter_context(tc.tile_pool(name="op", bufs=3))
    misc = ctx.enter_context(tc.tile_pool(name="misc", bufs=2))
    mm_psum = ctx.enter_context(tc.tile_pool(name="mmps", bufs=4, space="PSUM"))
    tr_psum = ctx.enter_context(tc.tile_pool(name="trps", bufs=1, space="PSUM"))
    rs_psum = ctx.enter_context(tc.tile_pool(name="rsps", bufs=3, space="PSUM"))

    # ---- constants ----
    wq_sb = consts.tile([P, ND, D], BF16)       # wqT[d,h] as [p, d_chunk, h]
    for c in range(ND):
        nc.sync.dma_start(out=wq_sb[:, c, :], in_=wqt[c * P:(c + 1) * P, :])
    ident = consts.tile([P, P], BF16)
    make_identity(nc, ident)
    ut_mask = consts.tile([P, P], BF16)         # 1 where q_loc >= k_loc
    make_upper_triangular(nc, ut_mask, val=1.0, diag=True)
    ones_col = consts.tile([P, 1], BF16)
    nc.vector.memset(ones_col, 1.0)
    ones_row = consts.tile([1, P], F32)
    nc.vector.memset(ones_row, 1.0)

    # ---- phase A: K = enc @ Wq.T in both layouts ----
    kt_sb = kt_pool.tile([P, NH, L], BF16)       # KT[h,k]: [p, h_chunk, k]
    kkh_sb = kkh_pool.tile([P, NK, D], BF16)     # Kkh[k,h]: [p, k_chunk, h]

    for kp in range(L // QW):                    # 4 k panels of 512
        enc_tile = enc_pool.tile([P, ND, QW], BF16, tag="enc")
        for c in range(ND):
            nc.sync.dma_start(
                out=enc_tile[:, c, :],
                in_=enct[c * P:(c + 1) * P, kp * QW:(kp + 1) * QW],
            )
        for ktl in range(QW // P):               # 4 k tiles of 128 in panel
            kt_g = kp * (QW // P) + ktl          # global k chunk index
            for hh in range(D // QW):            # 2 h halves of 512
                ps_k = mm_psum.tile([P, QW], F32, tag="mm")
                for c in range(ND):
                    nc.tensor.matmul(
                        ps_k,
                        lhsT=enc_tile[:, c, ktl * P:(ktl + 1) * P],
                        rhs=wq_sb[:, c, hh * QW:(hh + 1) * QW],
                        start=(c == 0),
                        stop=(c == ND - 1),
                    )
                nc.scalar.copy(
                    out=kkh_sb[:, kt_g, hh * QW:(hh + 1) * QW], in_=ps_k
                )
            for ch in range(NH):                 # transpose to KT[h,k]
                ps_t = tr_psum.tile([P, P], BF16, tag="tr")
                nc.tensor.transpose(
                    ps_t,
                    in_=kkh_sb[:, kt_g, ch * P:(ch + 1) * P],
                    identity=ident,
                )
                nc.vector.tensor_copy(
                    out=kt_sb[:, ch, kt_g * P:(kt_g + 1) * P], in_=ps_t
                )

    # ---- phase B: per q block of 512 ----
    for J in range(NJ):
        x_tile = x_pool.tile([P, ND, QW], BF16, tag="x")
        for c in range(ND):
            nc.sync.dma_start(
                out=x_tile[:, c, :],
                in_=xt[c * P:(c + 1) * P, J * QW:(J + 1) * QW],
            )
        # QT[h, q] for this block
        qt_sb = qt_pool.tile([P, NH, QW], BF16, tag="qt")
        for ch in range(NH):
            ps_q = mm_psum.tile([P, QW], F32, tag="mm")
            for c in range(ND):
                nc.tensor.matmul(
                    ps_q,
                    lhsT=wq_sb[:, c, ch * P:(ch + 1) * P],
                    rhs=x_tile[:, c, :],
                    start=(c == 0),
                    stop=(c == ND - 1),
                )
            nc.scalar.copy(out=qt_sb[:, ch, :], in_=ps_q)

        ncnk = 4 * J + 4                         # num k chunks with any valid q
        rs = rs_psum.tile([1, QW], F32, tag="rs")
        p_tiles = []
        col0s = []
        for c in range(ncnk):
            j = c - 4 * J                        # >=0 on diagonal chunks
            col0 = max(0, P * j)
            col0s.append(col0)
            ps_s = mm_psum.tile([P, QW], F32, tag="mm")
            for ch in range(NH):
                nc.tensor.matmul(
                    ps_s[:, col0:QW],
                    lhsT=kt_sb[:, ch, c * P:(c + 1) * P],
                    rhs=qt_sb[:, ch, col0:QW],
                    start=(ch == 0),
                    stop=(ch == NH - 1),
                )
            p_t = p_pool.tile([P, QW], BF16, tag="p")
            nc.scalar.activation(
                out=p_t[:, col0:QW],
                in_=ps_s[:, col0:QW],
                func=mybir.ActivationFunctionType.Exp,
                scale=float(SCALE),
            )
            if j >= 0:                           # causal mask on diagonal block
                nc.vector.tensor_mul(
                    out=p_t[:, col0:col0 + P],
                    in0=p_t[:, col0:col0 + P],
                    in1=ut_mask,
                )
            nc.tensor.matmul(                    # rowsum[q] += sum_k P[k,q]
                rs[0:1, col0:QW],
                lhsT=ones_col,
                rhs=p_t[:, col0:QW],
                start=(c == 0),
                stop=(c == ncnk - 1),
            )
            p_tiles.append(p_t)

        recip = misc.tile([1, QW], F32, tag="recip")
        nc.vector.reciprocal(out=recip, in_=rs[0:1, :])
        bc_ps = rs_psum.tile([P, QW], F32, tag="rs")
        nc.tensor.matmul(bc_ps, lhsT=ones_row, rhs=recip, start=True, stop=True)
        bcast = misc.tile([P, QW], F32, tag="bcast")
        nc.scalar.copy(out=bcast, in_=bc_ps)

        for ch in range(NH):                     # OT[h,q] = sum_k Kkh*P
            ps_o = mm_psum.tile([P, QW], F32, tag="mm")
            for c in range(ncnk):
                nc.tensor.matmul(
                    ps_o[:, col0s[c]:QW],
                    lhsT=kkh_sb[:, c, ch * P:(ch + 1) * P],
                    rhs=p_tiles[c][:, col0s[c]:QW],
                    start=(c == 0),
                    stop=(c == ncnk - 1),
                )
            o_sb = o_pool.tile([P, QW], F32, tag="o")
            nc.vector.tensor_mul(out=o_sb, in0=ps_o, in1=bcast)
            nc.sync.dma_start(out=outs[J][ch], in_=o_sb)
    ctx.close()


def _get_program():
    if "nc" not in _CACHED:
        _CACHED["nc"] = build_program()
    return _CACHED["nc"]


def kernel(enc_outputs: np.ndarray, x: np.ndarray, Wq: np.ndarray) -> np.ndarray:
    from concourse.bass_utils import run_bass_kernel_spmd

    nc = _get_program()
    bf16 = ml_dtypes.bfloat16
    wqt = np.ascontiguousarray(np.asarray(Wq, dtype=np.float32).T).astype(bf16)
    in_maps = []
    for b in range(B):
        in_maps.append({
            "xt": np.ascontiguousarray(np.asarray(x[b], np.float32).T).astype(bf16),
            "enct": np.ascontiguousarray(
                np.asarray(enc_outputs[b], np.float32).T).astype(bf16),
            "wqt": wqt,
        })
    res = run_bass_kernel_spmd(nc, in_maps, list(range(B)))
    _CACHED["last_result"] = res
    out = np.empty((B, L, D), dtype=np.float32)
    ot = np.empty((D, L), dtype=np.float32)
    for b in range(B):
        for J in range(NJ):
            for ch in range(NH):
                ot[ch * P:(ch + 1) * P, J * QW:(J + 1) * QW] = \
                    res.results[b][f"o_{J}_{ch}"]
        out[b] = ot.T
    return out



# revision 22
# speedup vs baseline: 1.0264x; 1.0264x over previous
"""Causal cross-attention (q=k=v projection) kernel for Trainium2, 8 cores.

Problem (hardcoded): B=8, L=2048, D=1024, fp32.
    q = x @ Wq.T; k = enc @ Wq.T; v = k
    s = causal(q @ k.T / sqrt(D)); out = softmax(s) @ v

Sharding: data-parallel over batch, one batch element per NeuronCore, Wq
replicated.  All on-chip compute in bf16 (fp32 PSUM accumulation); the host
pre-transposes inputs so every matmul has its contraction dim on partitions:

per core, with xT/encT/wqT host-prepared ([D,L]/[D,L]/[D,D] bf16):
  QT[h,q] = sum_d wqT[d,h] * xT[d,q]            (lhsT=wqT tile, rhs=xT tile)
  Kkh[k,h] = sum_d encT[d,k] * wqT[d,h]         (lhsT=encT tile, rhs=wqT)
  KT[h,k]  = DMA-xbar-transpose(Kkh)            (off the PE critical path)
  ST[k,q]  = sum_h KT[h,k] * QT[h,q]            (scores, transposed)
  P[k,q]   = exp(ST/32) * causal_mask           (no max-subtraction: |s|<~6)
  rowsum[q]= ones.T @ P                          (PE, accumulated over k chunks)
  OT[h,q]  = sum_k Kkh[k,h] * P[k,q]            (attn @ v, transposed)
  out_t    = OT * (1/rowsum)  broadcast over partitions
Host transposes out_t back to [L, D] per batch.
"""

import sys

for _p in ("/opt/trn_rl_repo",):
    if _p not in sys.path:
        sys.path.insert(0, _p)

import numpy as np
import ml_dtypes

import concourse.bass as bass
import concourse.tile as tile
from concourse import bacc, mybir
from concourse.masks import make_upper_triangular

B, L, D = 8, 2048, 1024
P = 128                    # partitions
ND = D // P                # 8 d-chunks
NH = D // P                # 8 h-chunks
NK = L // P                # 16 k-chunks
QW = 512                   # q block width
NJ = L // QW               # 4 q blocks
SCALE = 1.0 / np.sqrt(np.float32(D))   # 1/32

BF16 = mybir.dt.bfloat16
F32 = mybir.dt.float32

_CACHED = {}


def build_program():
    """Build the per-core Bass/Tile program (same for all 8 cores)."""
    # Bacc (not raw Bass): its compile() splits multi-sem waits into
    # EventSemaphore instructions — walrus encodes at most one wait per
    # instruction, so raw Bass programs with Tile-generated multi-waits
    # fail codegen ("Too many sync wait commands").
    nc = bacc.Bacc("TRN2", target_bir_lowering=False, debug=False, num_devices=B)

    xt = nc.declare_dram_parameter("xt", [D, L], BF16, isOutput=False).ap()
    enct = nc.declare_dram_parameter("enct", [D, L], BF16, isOutput=False).ap()
    wqt = nc.declare_dram_parameter("wqt", [D, D], BF16, isOutput=False).ap()
    # separate output tensor per (q-block, h-chunk) store: a single shared
    # output tensor makes Tile chain every store with a cross-queue WAW wait,
    # and 2-wait DMAs don't fit the direct2d descriptor (walrus error).
    outs = [
        [nc.declare_dram_parameter(f"o_{J}_{ch}", [P, QW], F32,
                                   isOutput=True).ap()
         for ch in range(NH)]
        for J in range(NJ)
    ]

    with tile.TileContext(nc) as tc:
        _emit(nc, tc, xt, enct, wqt, outs)
    nc.compile()
    _check_dma_waits(nc)
    return nc


def _check_dma_waits(nc):
    """HWDGE direct2d descriptors encode only ONE wait; multi-wait DMAs make
    walrus fail codegen. SWDGE (gpsimd queue) triggers run on the Pool
    sequencer where walrus can split waits, so only check HWDGE queues."""
    fn = nc.m.functions[0]
    bad = [
        (i.name, i.queue, [(w.ant_name, w.wait_value) for w in i.sync_info.on_wait])
        for blk in fn.blocks for i in blk.instructions
        if type(i).__name__ in ("InstDMACopy", "InstDmaTransposeAnt")
        and "DynamicHW" in (getattr(i, "queue", "") or "")
        and len(i.sync_info.on_wait) > 1
    ]
    assert not bad, f"multi-wait HWDGE DMAs: {bad[:4]} (+{len(bad)-4} more)"


def _emit(nc, tc, xt, enct, wqt, outs):
    from contextlib import ExitStack

    ctx = ExitStack()
    consts = ctx.enter_context(tc.tile_pool(name="consts", bufs=1))
    kt_pool = ctx.enter_context(tc.tile_pool(name="ktp", bufs=1))
    kkh_pool = ctx.enter_context(tc.tile_pool(name="kkhp", bufs=1))
    enc_pool = ctx.enter_context(tc.tile_pool(name="encp", bufs=3))
    enc0_pool = ctx.enter_context(tc.tile_pool(name="enc0p", bufs=1))
    x_pool = ctx.enter_context(tc.tile_pool(name="xp", bufs=4))
    qt_pool = ctx.enter_context(tc.tile_pool(name="qtp", bufs=2))
    p_pool = ctx.enter_context(tc.tile_pool(name="pp", bufs=18))
    o_pool = ctx.enter_context(tc.tile_pool(name="op", bufs=3))
    misc = ctx.enter_context(tc.tile_pool(name="misc", bufs=2))
    mm_psum = ctx.enter_context(tc.tile_pool(name="mmps", bufs=6, space="PSUM"))
    rs_psum = ctx.enter_context(tc.tile_pool(name="rsps", bufs=2, space="PSUM"))

    # ---- constants ----
    # wq split into 4 separate tiles [hh half][c group] — dependency tracking
    # is tile-granular, so separate tiles let the first matmuls start as soon
    # as their own quarter lands instead of waiting for the full 2 MB.
    wq_t = [
        [consts.tile([P, ND // 2, QW], BF16, name=f"wq_{hh}_{cg}")
         for cg in range(2)]
        for hh in range(2)
    ]

    def wq_ap(c, col0, col1):
        """wqT[d-chunk c, cols col0:col1] from the split tiles (one hh half)."""
        hh, base = (0, 0) if col1 <= QW else (1, QW)
        return wq_t[hh][c // 4][:, c % 4, col0 - base:col1 - base]

    ut_mask = consts.tile([P, P], BF16)         # 1 where q_loc >= k_loc
    make_upper_triangular(nc, ut_mask, val=1.0, diag=True)
    ones_col = consts.tile([P, 1], BF16)
    nc.vector.memset(ones_col, 1.0)
    ones_row = consts.tile([1, P], F32)
    nc.vector.memset(ones_row, 1.0)

    # ---- phase A: K = enc @ Wq.T in both layouts ----
    kt_sb = kt_pool.tile([P, NH, L], BF16)       # KT[h,k]: [p, h_chunk, k]
    kkh_sb = kkh_pool.tile([P, NK, D], BF16)     # Kkh[k,h]: [p, k_chunk, h]

    # 3D views of the DRAM inputs: (c p) rows -> [p, c, cols] access patterns
    # so a whole panel loads as ONE batched DMA (HWDGE per-DMA cost is high).
    wq3 = wqt.rearrange("(c p) d -> p c d", p=P)
    enc3 = enct.rearrange("(c p) l -> p c l", p=P)
    x3 = xt.rearrange("(c p) l -> p c l", p=P)

    # ---- DMA plan: ONE queue (SP), issued in deadline order ----
    # DMA bandwidth is a single shared resource and the HWDGE ring is FIFO,
    # so urgency order IS issue order.  Nothing else runs on the SP
    # sequencer, so a DMA's semaphore wait never blocks compute dispatch
    # (transposes on the ACT queue would stall the PSUM-freeing copies).
    enc_tiles = [enc_pool.tile([P, ND, QW], BF16, tag="enc", name=f"enc_{i}")
                 for i in range(1, 4)]
    enc0_s = [enc0_pool.tile([P, ND, P], BF16, tag=f"e0s{i}", name=f"enc0_s{i}")
              for i in range(4)]
    x_tiles = [x_pool.tile([P, ND, QW], BF16, tag="x", name=f"x_{i}")
               for i in range(NJ)]

    # panel 0 in per-ktl slice tiles so the first accumulation starts after
    # ~0.75 MB instead of the full 3 MB of wq+panel.
    nc.sync.dma_start(out=enc0_s[0], in_=enc3[:, :, 0:P])
    nc.sync.dma_start(out=wq_t[0][0], in_=wq3[:, 0:4, 0:QW])
    nc.sync.dma_start(out=wq_t[0][1], in_=wq3[:, 4:8, 0:QW])
    for s in range(1, 4):
        nc.sync.dma_start(out=enc0_s[s], in_=enc3[:, :, s * P:(s + 1) * P])
    nc.sync.dma_start(out=wq_t[1][0], in_=wq3[:, 0:4, QW:D])
    nc.sync.dma_start(out=wq_t[1][1], in_=wq3[:, 4:8, QW:D])

    for kp in range(L // QW):                    # 4 k panels of 512
        # prefetch next panel / x blocks ahead of this panel's transposes
        if kp + 1 < 4:
            nc.sync.dma_start(
                out=enc_tiles[kp],
                in_=enc3[:, :, (kp + 1) * QW:(kp + 2) * QW],
            )
        if kp >= 1:                              # x0 after p2, x1 after p3, ...
            nc.sync.dma_start(
                out=x_tiles[kp - 1], in_=x3[:, :, (kp - 1) * QW:kp * QW]
            )
        if kp == 3:
            nc.sync.dma_start(out=x_tiles[3], in_=x3[:, :, 3 * QW:4 * QW])
        for hh in range(D // QW):                # hh OUTER: wq high half has
            for ktl in range(QW // P):           # ~10us of slack to arrive
                kt_g = kp * (QW // P) + ktl      # global k chunk index
                enc_ap = (enc0_s[ktl][:, :, :] if kp == 0
                          else enc_tiles[kp - 1][:, :, ktl * P:(ktl + 1) * P])
                ps_k = mm_psum.tile([P, QW], F32, tag="mm")
                for c in range(ND):
                    nc.tensor.matmul(
                        ps_k,
                        lhsT=enc_ap[:, c, :],
                        rhs=wq_ap(c, hh * QW, (hh + 1) * QW),
                        start=(c == 0),
                        stop=(c == ND - 1),
                    )
                nc.scalar.copy(
                    out=kkh_sb[:, kt_g, hh * QW:(hh + 1) * QW], in_=ps_k
                )
        for ktl in range(QW // P):
            kt_g = kp * (QW // P) + ktl
            # KT[h,k] for all 8 h-chunks of this k chunk in ONE xbar-transpose
            # DMA (3D out AP -> 8 blocked 128x128 transposes).
            nc.sync.dma_start_transpose(
                out=kt_sb[:, :, kt_g * P:(kt_g + 1) * P],
                in_=kkh_sb[:, kt_g, :],
            )

    # ---- phase B: per q block of 512 ----
    for J in range(NJ):
        x_tile = x_tiles[J]
        # QT[h, q] for this block
        qt_sb = qt_pool.tile([P, NH, QW], BF16, tag="qt")
        for ch in range(NH):
            ps_q = mm_psum.tile([P, QW], F32, tag="mm")
            for c in range(ND):
                nc.tensor.matmul(
                    ps_q,
                    lhsT=wq_ap(c, ch * P, (ch + 1) * P),
                    rhs=x_tile[:, c, :],
                    start=(c == 0),
                    stop=(c == ND - 1),
                )
            nc.scalar.copy(out=qt_sb[:, ch, :], in_=ps_q)

        ncnk = 4 * J + 4                         # num k chunks with any valid q
        rs = rs_psum.tile([1, QW], F32, tag="rs")
        p_tiles = []
        col0s = []
        for c in range(ncnk):
            j = c - 4 * J                        # >=0 on diagonal chunks
            col0 = max(0, P * j)
            col0s.append(col0)
            ps_s = mm_psum.tile([P, QW], F32, tag="mm")
            for ch in range(NH):
                nc.tensor.matmul(
                    ps_s[:, col0:QW],
                    lhsT=kt_sb[:, ch, c * P:(c + 1) * P],
                    rhs=qt_sb[:, ch, col0:QW],
                    start=(ch == 0),
                    stop=(ch == NH - 1),
                )
            p_t = p_pool.tile([P, QW], BF16, tag="p")
            nc.scalar.activation(
                out=p_t[:, col0:QW],
                in_=ps_s[:, col0:QW],
                func=mybir.ActivationFunctionType.Exp,
                scale=float(SCALE),
            )
            if j >= 0:                           # causal mask on diagonal block
                nc.vector.tensor_mul(
                    out=p_t[:, col0:col0 + P],
                    in0=p_t[:, col0:col0 + P],
                    in1=ut_mask,
                )
            nc.tensor.matmul(                    # rowsum[q] += sum_k P[k,q]
                rs[0:1, col0:QW],
                lhsT=ones_col,
                rhs=p_t[:, col0:QW],
                start=(c == 0),
                stop=(c == ncnk - 1),
            )
            p_tiles.append(p_t)

        recip = misc.tile([1, QW], F32, tag="recip")
        nc.vector.reciprocal(out=recip, in_=rs[0:1, :])
        bc_ps = rs_psum.tile([P, QW], F32, tag="rs")
        nc.tensor.matmul(bc_ps, lhsT=ones_row, rhs=recip, start=True, stop=True)
        bcast = misc.tile([P, QW], F32, tag="bcast")
        nc.scalar.copy(out=bcast, in_=bc_ps)

        for ch in range(NH):                     # OT[h,q] = sum_k Kkh*P
            ps_o = mm_psum.tile([P, QW], F32, tag="mm")
            for c in range(ncnk):
                nc.tensor.matmul(
                    ps_o[:, col0s[c]:QW],
                    lhsT=kkh_sb[:, c, ch * P:(ch + 1) * P],
                    rhs=p_tiles[c][:, col0s[c]:QW],
                    start=(c == 0),
                    stop=(c == ncnk - 1),
                )
            o_sb = o_pool.tile([P, QW], F32, tag="o")
            nc.vector.tensor_mul(out=o_sb, in0=ps_o, in1=bcast)
            nc.sync.dma_start(out=outs[J][ch], in_=o_sb)
    ctx.close()


def _get_program():
    if "nc" not in _CACHED:
        _CACHED["nc"] = build_program()
    return _CACHED["nc"]


def kernel(enc_outputs: np.ndarray, x: np.ndarray, Wq: np.ndarray) -> np.ndarray:
    from concourse.bass_utils import run_bass_kernel_spmd

    nc = _get_program()
    bf16 = ml_dtypes.bfloat16
    wqt = np.ascontiguousarray(np.asarray(Wq, dtype=np.float32).T).astype(bf16)
    in_maps = []
    for b in range(B):
        in_maps.append({
            "xt": np.ascontiguousarray(np.asarray(x[b], np.float32).T).astype(bf16),
            "enct": np.ascontiguousarray(
                np.asarray(enc_outputs[b], np.float32).T).astype(bf16),
            "wqt": wqt,
        })
    res = run_bass_kernel_spmd(nc, in_maps, list(range(B)))
    _CACHED["last_result"] = res
    out = np.empty((B, L, D), dtype=np.float32)
    ot = np.empty((D, L), dtype=np.float32)
    for b in range(B):
        for J in range(NJ):
            for ch in range(NH):
                ot[ch * P:(ch + 1) * P, J * QW:(J + 1) * QW] = \
                    res.results[b][f"o_{J}_{ch}"]
        out[b] = ot.T
    return out



# revision 23
# speedup vs baseline: 7330.4948x; 7141.6745x over previous
"""Causal cross-attention (q=k=v projection) kernel for Trainium2, 8 cores.

Problem (hardcoded): B=8, L=2048, D=1024, fp32.
    q = x @ Wq.T; k = enc @ Wq.T; v = k
    s = causal(q @ k.T / sqrt(D)); out = softmax(s) @ v

Sharding: data-parallel over batch, one batch element per NeuronCore, Wq
replicated.  All on-chip compute in bf16 (fp32 PSUM accumulation); the host
pre-transposes inputs so every matmul has its contraction dim on partitions:

per core, with xT/encT/wqT host-prepared ([D,L]/[D,L]/[D,D] bf16):
  QT[h,q] = sum_d wqT[d,h] * xT[d,q]            (lhsT=wqT tile, rhs=xT tile)
  Kkh[k,h] = sum_d encT[d,k] * wqT[d,h]         (lhsT=encT tile, rhs=wqT)
  KT[h,k]  = DMA-xbar-transpose(Kkh)            (off the PE critical path)
  ST[k,q]  = sum_h KT[h,k] * QT[h,q]            (scores, transposed)
  P[k,q]   = exp(ST/32) * causal_mask           (no max-subtraction: |s|<~6)
  rowsum[q]= ones.T @ P                          (PE, accumulated over k chunks)
  OT[h,q]  = sum_k Kkh[k,h] * P[k,q]            (attn @ v, transposed)
  out_t    = OT * (1/rowsum)  broadcast over partitions
Host transposes out_t back to [L, D] per batch.
"""

import sys

for _p in ("/opt/trn_rl_repo",):
    if _p not in sys.path:
        sys.path.insert(0, _p)

import numpy as np
import ml_dtypes

import concourse.bass as bass
import concourse.tile as tile
from concourse import bacc, mybir
from concourse.masks import make_upper_triangular

B, L, D = 8, 2048, 1024
P = 128                    # partitions
ND = D // P                # 8 d-chunks
NH = D // P                # 8 h-chunks
NK = L // P                # 16 k-chunks
QW = 512                   # q block width
NJ = L // QW               # 4 q blocks
SCALE = 1.0 / np.sqrt(np.float32(D))   # 1/32

BF16 = mybir.dt.bfloat16
F32 = mybir.dt.float32

_CACHED = {}


def build_program():
    """Build the per-core Bass/Tile program (same for all 8 cores)."""
    # Bacc (not raw Bass): its compile() splits multi-sem waits into
    # EventSemaphore instructions — walrus encodes at most one wait per
    # instruction, so raw Bass programs with Tile-generated multi-waits
    # fail codegen ("Too many sync wait commands").
    nc = bacc.Bacc("TRN2", target_bir_lowering=False, debug=False, num_devices=B)

    xt = nc.declare_dram_parameter("xt", [D, L], BF16, isOutput=False).ap()
    enct = nc.declare_dram_parameter("enct", [D, L], BF16, isOutput=False).ap()
    wqt = nc.declare_dram_parameter("wqt", [D, D], BF16, isOutput=False).ap()
    # separate output tensor per (q-block, h-chunk) store: a single shared
    # output tensor makes Tile chain every store with a cross-queue WAW wait,
    # and 2-wait DMAs don't fit the direct2d descriptor (walrus error).
    outs = [
        [nc.declare_dram_parameter(f"o_{J}_{ch}", [P, QW], F32,
                                   isOutput=True).ap()
         for ch in range(NH)]
        for J in range(NJ)
    ]

    with tile.TileContext(nc) as tc:
        _emit(nc, tc, xt, enct, wqt, outs)
    nc.compile()
    _check_dma_waits(nc)
    return nc


def _check_dma_waits(nc):
    """HWDGE direct2d descriptors encode only ONE wait; multi-wait DMAs make
    walrus fail codegen. SWDGE (gpsimd queue) triggers run on the Pool
    sequencer where walrus can split waits, so only check HWDGE queues."""
    fn = nc.m.functions[0]
    bad = [
        (i.name, i.queue, [(w.ant_name, w.wait_value) for w in i.sync_info.on_wait])
        for blk in fn.blocks for i in blk.instructions
        if type(i).__name__ in ("InstDMACopy", "InstDmaTransposeAnt")
        and "DynamicHW" in (getattr(i, "queue", "") or "")
        and len(i.sync_info.on_wait) > 1
    ]
    assert not bad, f"multi-wait HWDGE DMAs: {bad[:4]} (+{len(bad)-4} more)"


def _emit(nc, tc, xt, enct, wqt, outs):
    from contextlib import ExitStack

    ctx = ExitStack()
    consts = ctx.enter_context(tc.tile_pool(name="consts", bufs=1))
    kt_pool = ctx.enter_context(tc.tile_pool(name="ktp", bufs=1))
    kkh_pool = ctx.enter_context(tc.tile_pool(name="kkhp", bufs=1))
    enc_pool = ctx.enter_context(tc.tile_pool(name="encp", bufs=3))
    enc0_pool = ctx.enter_context(tc.tile_pool(name="enc0p", bufs=1))
    x_pool = ctx.enter_context(tc.tile_pool(name="xp", bufs=4))
    qt_pool = ctx.enter_context(tc.tile_pool(name="qtp", bufs=2))
    p_pool = ctx.enter_context(tc.tile_pool(name="pp", bufs=18))
    o_pool = ctx.enter_context(tc.tile_pool(name="op", bufs=3))
    misc = ctx.enter_context(tc.tile_pool(name="misc", bufs=2))
    acc_pool = ctx.enter_context(tc.tile_pool(name="accp", bufs=2))
    mm_psum = ctx.enter_context(tc.tile_pool(name="mmps", bufs=6, space="PSUM"))
    rs_psum = ctx.enter_context(tc.tile_pool(name="rsps", bufs=2, space="PSUM"))

    # ---- constants ----
    # wq split into 4 separate tiles [hh half][c group] — dependency tracking
    # is tile-granular, so separate tiles let the first matmuls start as soon
    # as their own quarter lands instead of waiting for the full 2 MB.
    wq_t = [
        [consts.tile([P, ND // 2, QW], BF16, name=f"wq_{hh}_{cg}")
         for cg in range(2)]
        for hh in range(2)
    ]

    def wq_ap(c, col0, col1):
        """wqT[d-chunk c, cols col0:col1] from the split tiles (one hh half)."""
        hh, base = (0, 0) if col1 <= QW else (1, QW)
        return wq_t[hh][c // 4][:, c % 4, col0 - base:col1 - base]

    ut_mask = consts.tile([P, P], BF16)         # 1 where q_loc >= k_loc
    make_upper_triangular(nc, ut_mask, val=1.0, diag=True)
    ones_col = consts.tile([P, 1], F32)
    nc.vector.memset(ones_col, 1.0)
    ones_row = consts.tile([1, P], F32)
    nc.vector.memset(ones_row, 1.0)

    # ---- phase A: K = enc @ Wq.T in both layouts ----
    kt_sb = kt_pool.tile([P, NH, L], BF16)       # KT[h,k]: [p, h_chunk, k]
    kkh_sb = kkh_pool.tile([P, NK, D], BF16)     # Kkh[k,h]: [p, k_chunk, h]

    # 3D views of the DRAM inputs: (c p) rows -> [p, c, cols] access patterns
    # so a whole panel loads as ONE batched DMA (HWDGE per-DMA cost is high).
    wq3 = wqt.rearrange("(c p) d -> p c d", p=P)
    enc3 = enct.rearrange("(c p) l -> p c l", p=P)
    x3 = xt.rearrange("(c p) l -> p c l", p=P)

    # ---- DMA plan: ONE queue (SP), issued in deadline order ----
    # DMA bandwidth is a single shared resource and the HWDGE ring is FIFO,
    # so urgency order IS issue order.  Nothing else runs on the SP
    # sequencer, so a DMA's semaphore wait never blocks compute dispatch
    # (transposes on the ACT queue would stall the PSUM-freeing copies).
    enc_tiles = [enc_pool.tile([P, ND, QW], BF16, tag="enc", name=f"enc_{i}")
                 for i in range(1, 4)]
    enc0_s = [enc0_pool.tile([P, ND, P], BF16, tag=f"e0s{i}", name=f"enc0_s{i}")
              for i in range(4)]
    x_tiles = [x_pool.tile([P, ND, QW], BF16, tag="x", name=f"x_{i}")
               for i in range(NJ)]

    # panel 0 in per-ktl slice tiles so the first accumulation starts after
    # ~0.75 MB instead of the full 3 MB of wq+panel.
    nc.sync.dma_start(out=enc0_s[0], in_=enc3[:, :, 0:P])
    nc.sync.dma_start(out=wq_t[0][0], in_=wq3[:, 0:4, 0:QW])
    nc.sync.dma_start(out=wq_t[0][1], in_=wq3[:, 4:8, 0:QW])
    for s in range(1, 4):
        nc.sync.dma_start(out=enc0_s[s], in_=enc3[:, :, s * P:(s + 1) * P])
    nc.sync.dma_start(out=wq_t[1][0], in_=wq3[:, 0:4, QW:D])
    nc.sync.dma_start(out=wq_t[1][1], in_=wq3[:, 4:8, QW:D])

    for kp in range(L // QW):                    # 4 k panels of 512
        # prefetch next panel / x blocks ahead of this panel's transposes
        if kp + 1 < 4:
            nc.sync.dma_start(
                out=enc_tiles[kp],
                in_=enc3[:, :, (kp + 1) * QW:(kp + 2) * QW],
            )
        if kp >= 1:                              # x0 after p2, x1 after p3, ...
            nc.sync.dma_start(
                out=x_tiles[kp - 1], in_=x3[:, :, (kp - 1) * QW:kp * QW]
            )
        if kp == 3:
            nc.sync.dma_start(out=x_tiles[3], in_=x3[:, :, 3 * QW:4 * QW])
        for hh in range(D // QW):                # hh OUTER: wq high half has
            for ktl in range(QW // P):           # ~10us of slack to arrive
                kt_g = kp * (QW // P) + ktl      # global k chunk index
                enc_ap = (enc0_s[ktl][:, :, :] if kp == 0
                          else enc_tiles[kp - 1][:, :, ktl * P:(ktl + 1) * P])
                ps_k = mm_psum.tile([P, QW], F32, tag="mm")
                for c in range(ND):
                    nc.tensor.matmul(
                        ps_k,
                        lhsT=enc_ap[:, c, :],
                        rhs=wq_ap(c, hh * QW, (hh + 1) * QW),
                        start=(c == 0),
                        stop=(c == ND - 1),
                    )
                nc.scalar.copy(
                    out=kkh_sb[:, kt_g, hh * QW:(hh + 1) * QW], in_=ps_k
                )
        for ktl in range(QW // P):
            kt_g = kp * (QW // P) + ktl
            # KT[h,k] for all 8 h-chunks of this k chunk in ONE xbar-transpose
            # DMA (3D out AP -> 8 blocked 128x128 transposes).
            nc.sync.dma_start_transpose(
                out=kt_sb[:, :, kt_g * P:(kt_g + 1) * P],
                in_=kkh_sb[:, kt_g, :],
            )

    # ---- phase B: per q block of 512 ----
    for J in range(NJ):
        x_tile = x_tiles[J]
        # QT[h, q] for this block
        qt_sb = qt_pool.tile([P, NH, QW], BF16, tag="qt")
        for ch in range(NH):
            ps_q = mm_psum.tile([P, QW], F32, tag="mm")
            for c in range(ND):
                nc.tensor.matmul(
                    ps_q,
                    lhsT=wq_ap(c, ch * P, (ch + 1) * P),
                    rhs=x_tile[:, c, :],
                    start=(c == 0),
                    stop=(c == ND - 1),
                )
            nc.scalar.copy(out=qt_sb[:, ch, :], in_=ps_q)

        ncnk = 4 * J + 4                         # num k chunks with any valid q
        rs = rs_psum.tile([1, QW], F32, tag="rs")
        acc = acc_pool.tile([P, QW], F32, tag="acc")
        p_tiles = []
        col0s = []
        for c in range(ncnk):
            j = c - 4 * J                        # >=0 on diagonal chunks
            col0 = max(0, P * j)
            col0s.append(col0)
            ps_s = mm_psum.tile([P, QW], F32, tag="mm")
            for ch in range(NH):
                nc.tensor.matmul(
                    ps_s[:, col0:QW],
                    lhsT=kt_sb[:, ch, c * P:(c + 1) * P],
                    rhs=qt_sb[:, ch, col0:QW],
                    start=(ch == 0),
                    stop=(ch == NH - 1),
                )
            p_t = p_pool.tile([P, QW], BF16, tag="p")
            nc.scalar.activation(
                out=p_t[:, col0:QW],
                in_=ps_s[:, col0:QW],
                func=mybir.ActivationFunctionType.Exp,
                scale=float(SCALE),
            )
            if j >= 0:                           # causal mask on diagonal block
                nc.vector.tensor_mul(
                    out=p_t[:, col0:col0 + P],
                    in0=p_t[:, col0:col0 + P],
                    in1=ut_mask,
                )
            if c == 0:                           # running sum on DVE frees
                nc.vector.tensor_copy(out=acc, in_=p_t)   # 36 PE matmuls
            else:
                nc.vector.tensor_add(
                    out=acc[:, col0:QW],
                    in0=acc[:, col0:QW],
                    in1=p_t[:, col0:QW],
                )
            p_tiles.append(p_t)

        nc.tensor.matmul(rs, lhsT=ones_col, rhs=acc, start=True, stop=True)
        recip = misc.tile([1, QW], F32, tag="recip")
        nc.vector.reciprocal(out=recip, in_=rs[0:1, :])
        bc_ps = rs_psum.tile([P, QW], F32, tag="rs")
        nc.tensor.matmul(bc_ps, lhsT=ones_row, rhs=recip, start=True, stop=True)
        bcast = misc.tile([P, QW], F32, tag="bcast")
        nc.scalar.copy(out=bcast, in_=bc_ps)

        for ch in range(NH):                     # OT[h,q] = sum_k Kkh*P
            ps_o = mm_psum.tile([P, QW], F32, tag="mm")
            for c in range(ncnk):
                nc.tensor.matmul(
                    ps_o[:, col0s[c]:QW],
                    lhsT=kkh_sb[:, c, ch * P:(ch + 1) * P],
                    rhs=p_tiles[c][:, col0s[c]:QW],
                    start=(c == 0),
                    stop=(c == ncnk - 1),
                )
            o_sb = o_pool.tile([P, QW], F32, tag="o")
            nc.vector.tensor_mul(out=o_sb, in0=ps_o, in1=bcast)
            nc.sync.dma_start(out=outs[J][ch], in_=o_sb)
    ctx.close()


def _get_program():
    if "nc" not in _CACHED:
        _CACHED["nc"] = build_program()
    return _CACHED["nc"]


def kernel(enc_outputs: np.ndarray, x: np.ndarray, Wq: np.ndarray) -> np.ndarray:
    from concourse.bass_utils import run_bass_kernel_spmd

    nc = _get_program()
    bf16 = ml_dtypes.bfloat16
    wqt = np.ascontiguousarray(np.asarray(Wq, dtype=np.float32).T).astype(bf16)
    in_maps = []
    for b in range(B):
        in_maps.append({
            "xt": np.ascontiguousarray(np.asarray(x[b], np.float32).T).astype(bf16),
            "enct": np.ascontiguousarray(
                np.asarray(enc_outputs[b], np.float32).T).astype(bf16),
            "wqt": wqt,
        })
    res = run_bass_kernel_spmd(nc, in_maps, list(range(B)))
    _CACHED["last_result"] = res
    out = np.empty((B, L, D), dtype=np.float32)
    ot = np.empty((D, L), dtype=np.float32)
    for b in range(B):
        for J in range(NJ):
            for ch in range(NH):
                ot[ch * P:(ch + 1) * P, J * QW:(J + 1) * QW] = \
                    res.results[b][f"o_{J}_{ch}"]
        out[b] = ot.T
    return out

